# revision 5
# baseline (speedup 1.0000x reference)
"""Trainium2 Bass kernel for a diagonal-A linear dynamical system (LDS).

    Bu = inputs @ B            [B, T, S]
    h_t = h_{t-1} * A + Bu_t   (scan over T, diagonal A)
    y_t = h_t @ C              [B, T, O]

Shapes: inputs [16, 4096, 256], A [256], B [256, 256], C [256, 256],
h0 [256]; all float32.

Sharding: data-parallel over batch across 8 NeuronCores (2 batches per
core); A/B/C/h0 replicated.

Per-core dataflow (all tiles 128-partition):
  1. DMA u supertile [128t, 4sub, 256i] (natural layout, contiguous i).
  2. PE transpose 128x128 blocks -> uT [128i, 512t] in PSUM.
  3. ACT copies uT PSUM->SBUF (dtype knob applies here).
  4. PE matmul BuT[s, t] = B^T @ uT accumulated over i-halves into PSUM.
  5. DVE tensor_tensor_scan along t: state = A*state + Bu (exactly the
     recurrence; fp32 internal state), chained across supertiles via
     initial=prev last column. Output hT in SBUF.
  6. PE matmul y[t, o] = hT.T @ C (hT slices are the stationary operand).
  7. ACT copy y PSUM->SBUF, DMA out.
"""

import numpy as np

import concourse.bacc as bacc
import concourse.bass as bass
import concourse.mybir as mybir
import concourse.tile as tile
from concourse import bass_utils
from concourse.masks import make_identity

BATCH, T, D = 16, 4096, 256
NCORES = 8
BLOC = BATCH // NCORES  # batches per core
TT = 512                # time supertile
NSUB = TT // 128        # 128-row subtiles per supertile
NJ = T // TT            # supertiles per sequence
F32 = mybir.dt.float32

# Matmul operand dtype knob: float32 (exact), float32r (fast fp32 mode),
# bfloat16 (fastest, lossy).
MM_DT = mybir.dt.float32

_CACHE: dict = {}


def _build_nc():
    nc = bacc.Bacc(trn_type="TRN2", target_bir_lowering=False)

    u = nc.dram_tensor("u", [BLOC, T, D], F32, kind="ExternalInput")
    Ad = nc.dram_tensor("A", [128, 2], F32, kind="ExternalInput")      # [s%128, s//128]
    Bd = nc.dram_tensor("B", [2, 128, D], F32, kind="ExternalInput")   # [ihalf, i, s]
    Cd = nc.dram_tensor("C", [2, 128, D], F32, kind="ExternalInput")   # [shalf, s, o]
    h0d = nc.dram_tensor("h0", [128, 2], F32, kind="ExternalInput")
    y = nc.dram_tensor("y", [BLOC, T, D], F32, kind="ExternalOutput")

    # t = j*TT + sub*128 + p
    u_r = u[:].rearrange("b (j s p) i -> b j p s i", p=128, s=NSUB)
    y_r = y[:].rearrange("b (j s p) o -> b j p s o", p=128, s=NSUB)

    mult = mybir.AluOpType.mult
    add = mybir.AluOpType.add

    with tile.TileContext(nc) as tc:
        with (
            tc.tile_pool(name="const", bufs=1) as const,
            tc.tile_pool(name="sbuf", bufs=3) as sbuf,
            tc.tile_pool(name="hpool", bufs=1) as hpool,
            tc.tile_pool(name="ps_ut", bufs=2, space="PSUM") as ps_ut,
            tc.tile_pool(name="ps_bu", bufs=2, space="PSUM") as ps_bu,
            tc.tile_pool(name="ps_y", bufs=3, space="PSUM") as ps_y,
        ):
            # --- constants ---
            ident = const.tile([128, 128], F32, name="ident")
            make_identity(nc, ident)

            A_col = const.tile([128, 2], F32, name="A_col")
            nc.sync.dma_start(A_col, Ad[:])
            h0c = const.tile([128, 2], F32, name="h0c")
            nc.sync.dma_start(h0c, h0d[:])

            ones = const.tile([128, TT], F32, name="ones")
            nc.vector.memset(ones, 1.0)
            A_bc = const.tile([128, 2, TT], F32, name="A_bc")
            for m in range(2):
                nc.scalar.mul(A_bc[:, m], ones, mul=A_col[:, m : m + 1])

            B_sb = const.tile([128, 2, D], MM_DT, name="B_sb")
            C_sb = const.tile([128, 2, D], MM_DT, name="C_sb")
            dma_w = nc.sync.dma_start if MM_DT == F32 else nc.gpsimd.dma_start
            for k in range(2):
                dma_w(B_sb[:, k], Bd[k])
                dma_w(C_sb[:, k], Cd[k])

            # hidden states, [128s, b, mhalf, t]; persistent
            h_dt = MM_DT if MM_DT == mybir.dt.bfloat16 else F32
            hT = hpool.tile([128, BLOC, 2, T], h_dt, name="hT")

            for b in range(BLOC):
                for j in range(NJ):
                    u_t = sbuf.tile([128, NSUB, D], F32, tag="u_t", name="u_t")
                    nc.sync.dma_start(u_t, u_r[b, j])

                    uTs = []
                    for k in range(2):
                        uT_ps = ps_ut.tile([128, TT], F32, tag="uT_ps", name="uT_ps")
                        for s_ in range(NSUB):
                            nc.tensor.transpose(
                                uT_ps[:, s_ * 128 : (s_ + 1) * 128],
                                u_t[:, s_, k * 128 : (k + 1) * 128],
                                ident,
                            )
                        uT_sb = sbuf.tile([128, TT], MM_DT, tag="uT_sb", bufs=4,
                                          name="uT_sb")
                        nc.scalar.copy(uT_sb, uT_ps)
                        uTs.append(uT_sb)

                    for m in range(2):
                        bu_ps = ps_bu.tile([128, TT], F32, tag="bu_ps", name="bu_ps")
                        for k in range(2):
                            nc.tensor.matmul(
                                bu_ps,
                                B_sb[:, k, m * 128 : (m + 1) * 128],
                                uTs[k],
                                start=(k == 0),
                                stop=(k == 1),
                            )
                        init = (
                            h0c[:, m : m + 1]
                            if j == 0
                            else hT[:, b, m, j * TT - 1 : j * TT]
                        )
                        nc.vector.tensor_tensor_scan(
                            hT[:, b, m, j * TT : (j + 1) * TT],
                            A_bc[:, m],
                            bu_ps,
                            init,
                            op0=mult,
                            op1=add,
                        )

                    y_sb = sbuf.tile([128, NSUB, D], F32, tag="y_sb", name="y_sb")
                    for s_ in range(NSUB):
                        t0 = j * TT + s_ * 128
                        y_ps = ps_y.tile([128, D], F32, tag="y_ps", name="y_ps")
                        for k in range(2):
                            nc.tensor.matmul(
                                y_ps,
                                hT[:, b, k, t0 : t0 + 128],
                                C_sb[:, k],
                                start=(k == 0),
                                stop=(k == 1),
                            )
                        nc.scalar.copy(y_sb[:, s_], y_ps)
                    nc.sync.dma_start(y_r[b, j], y_sb)

    nc.compile()
    return nc


def _get_nc():
    if "nc" not in _CACHE:
        _CACHE["nc"] = _build_nc()
    return _CACHE["nc"]


def make_in_maps(inputs, A, B, C, h0):
    u = np.ascontiguousarray(np.asarray(inputs, dtype=np.float32))
    A2 = np.ascontiguousarray(np.asarray(A, np.float32).reshape(2, 128).T)
    h02 = np.ascontiguousarray(np.asarray(h0, np.float32).reshape(2, 128).T)
    Br = np.ascontiguousarray(np.asarray(B, np.float32).reshape(2, 128, D))
    Cr = np.ascontiguousarray(np.asarray(C, np.float32).reshape(2, 128, D))
    return [
        {
            "u": np.ascontiguousarray(u[c * BLOC : (c + 1) * BLOC]),
            "A": A2,
            "B": Br,
            "C": Cr,
            "h0": h02,
        }
        for c in range(NCORES)
    ]


def kernel(inputs, A, B, C, h0, _trace=False):
    nc = _get_nc()
    in_maps = make_in_maps(inputs, A, B, C, h0)
    res = bass_utils.run_bass_kernel_spmd(
        nc, in_maps, core_ids=list(range(NCORES)), trace=_trace
    )
    out = np.concatenate([r["y"] for r in res.results], axis=0)
    if _trace:
        _CACHE["last_result"] = res
    return out


# revision 12
# speedup vs baseline: 1.7307x; 1.7307x over previous
"""Trainium2 Bass kernel for a diagonal-A linear dynamical system (LDS).

    Bu = inputs @ B            [B, T, S]
    h_t = h_{t-1} * A + Bu_t   (scan over T, diagonal A)
    y_t = h_t @ C              [B, T, O]

Shapes: inputs [16, 4096, 256], A [256], B [256, 256], C [256, 256],
h0 [256]; all float32.

Sharding: data-parallel over batch across 8 NeuronCores (2 batches per
core); A/B/C/h0 replicated.

Per-core dataflow (all tiles 128-partition):
  1. DMA u supertile [128t, 4sub, 256i] (natural layout, contiguous i).
  2. PE transpose 128x128 blocks -> uT [128i, 512t] in PSUM.
  3. ACT copies uT PSUM->SBUF (dtype knob applies here).
  4. PE matmul BuT[s, t] = B^T @ uT accumulated over i-halves into PSUM.
  5. DVE tensor_tensor_scan along t: state = A*state + Bu (exactly the
     recurrence; fp32 internal state), chained across supertiles via
     initial=prev last column. Output hT in SBUF.
  6. PE matmul y[t, o] = hT.T @ C (hT slices are the stationary operand).
  7. ACT copy y PSUM->SBUF, DMA out.
"""

import numpy as np

import concourse.bacc as bacc
import concourse.bass as bass
import concourse.mybir as mybir
import concourse.tile as tile
from concourse import bass_utils
from concourse.masks import make_identity

BATCH, T, D = 16, 4096, 256
NCORES = 8
BLOC = BATCH // NCORES  # batches per core
TT = 512                # time supertile
NSUB = TT // 128        # 128-row subtiles per supertile
NJ = T // TT            # supertiles per sequence
F32 = mybir.dt.float32

# Matmul operand dtype knob: float32 (exact, 4 cyc/row), float32r (fp32
# data, 1 cyc/row at N>=256), bfloat16 (1 cyc/row, lossy).
MM_DT = mybir.dt.float32r

_CACHE: dict = {}


def _build_nc():
    nc = bacc.Bacc(trn_type="TRN2", target_bir_lowering=False)

    u = nc.dram_tensor("u", [BLOC, T, D], F32, kind="ExternalInput")
    Ad = nc.dram_tensor("A", [128, 2], F32, kind="ExternalInput")      # [s%128, s//128]
    Bd = nc.dram_tensor("B", [2, 128, D], MM_DT, kind="ExternalInput")  # [ihalf, i, s]
    Cd = nc.dram_tensor("C", [2, 128, D], MM_DT, kind="ExternalInput")  # [shalf, s, o]
    h0d = nc.dram_tensor("h0", [128, 2], F32, kind="ExternalInput")
    y = nc.dram_tensor("y", [BLOC, T, D], F32, kind="ExternalOutput")

    # t = j*TT + sub*128 + p
    u_r = u[:].rearrange("b (j s p) i -> b j p s i", p=128, s=NSUB)
    y_r = y[:].rearrange("b (j s p) o -> b j p s o", p=128, s=NSUB)

    mult = mybir.AluOpType.mult
    add = mybir.AluOpType.add

    with tile.TileContext(nc) as tc:
        with (
            tc.tile_pool(name="const", bufs=1) as const,
            tc.tile_pool(name="sbuf", bufs=3) as sbuf,
            tc.tile_pool(name="hpool", bufs=1) as hpool,
            tc.tile_pool(name="ps_ut", bufs=2, space="PSUM") as ps_ut,
            tc.tile_pool(name="ps_bu", bufs=2, space="PSUM") as ps_bu,
            tc.tile_pool(name="ps_y", bufs=3, space="PSUM") as ps_y,
        ):
            # --- constants ---
            ident = const.tile([128, 128], F32, name="ident")
            make_identity(nc, ident)

            A_col = const.tile([128, 2], F32, name="A_col")
            nc.sync.dma_start(A_col, Ad[:])
            h0c = const.tile([128, 2], F32, name="h0c")
            nc.sync.dma_start(h0c, h0d[:])

            ones = const.tile([128, TT], F32, name="ones")
            nc.vector.memset(ones, 1.0)
            A_bc = const.tile([128, 2, TT], F32, name="A_bc")
            for m in range(2):
                nc.scalar.mul(A_bc[:, m], ones, mul=A_col[:, m : m + 1])

            B_sb = const.tile([128, 2, D], MM_DT, name="B_sb")
            C_sb = const.tile([128, 2, D], MM_DT, name="C_sb")
            dma_w = (
                nc.gpsimd.dma_start
                if MM_DT == mybir.dt.bfloat16
                else nc.sync.dma_start
            )
            for k in range(2):
                dma_w(B_sb[:, k], Bd[k])
                dma_w(C_sb[:, k], Cd[k])

            # hidden states, [128s, b, mhalf, t]; persistent
            hT = hpool.tile([128, BLOC, 2, T], MM_DT, name="hT")

            for b in range(BLOC):
                for j in range(NJ):
                    u_t = sbuf.tile([128, NSUB, D], F32, tag="u_t", name="u_t")
                    nc.sync.dma_start(u_t, u_r[b, j])

                    uTs = []
                    for k in range(2):
                        uT_ps = ps_ut.tile([128, TT], F32, tag="uT_ps", name="uT_ps")
                        for s_ in range(NSUB):
                            nc.tensor.transpose(
                                uT_ps[:, s_ * 128 : (s_ + 1) * 128],
                                u_t[:, s_, k * 128 : (k + 1) * 128],
                                ident,
                            )
                        uT_sb = sbuf.tile([128, TT], MM_DT, tag="uT_sb", bufs=4,
                                          name="uT_sb")
                        nc.scalar.copy(uT_sb, uT_ps)
                        uTs.append(uT_sb)

                    for m in range(2):
                        bu_ps = ps_bu.tile([128, TT], F32, tag="bu_ps", name="bu_ps")
                        for k in range(2):
                            nc.tensor.matmul(
                                bu_ps,
                                B_sb[:, k, m * 128 : (m + 1) * 128],
                                uTs[k],
                                start=(k == 0),
                                stop=(k == 1),
                            )
                        init = (
                            h0c[:, m : m + 1]
                            if j == 0
                            else hT[:, b, m, j * TT - 1 : j * TT]
                        )
                        nc.vector.tensor_tensor_scan(
                            hT[:, b, m, j * TT : (j + 1) * TT],
                            A_bc[:, m],
                            bu_ps,
                            init,
                            op0=mult,
                            op1=add,
                        )

                    y_sb = sbuf.tile([128, NSUB * D], F32, tag="y_sb", name="y_sb")
                    for half in range(NSUB // 2):
                        y_ps = ps_y.tile([128, 2 * D], F32, tag="y_ps", name="y_ps")
                        for i in range(2):
                            s_ = half * 2 + i
                            t0 = j * TT + s_ * 128
                            for k in range(2):
                                nc.tensor.matmul(
                                    y_ps[:, i * D : (i + 1) * D],
                                    hT[:, b, k, t0 : t0 + 128],
                                    C_sb[:, k],
                                    start=(k == 0),
                                    stop=(k == 1),
                                )
                        nc.scalar.copy(
                            y_sb[:, half * 2 * D : (half + 1) * 2 * D], y_ps
                        )
                    nc.sync.dma_start(
                        y_r[b, j], y_sb.rearrange("p (s o) -> p s o", s=NSUB)
                    )

    nc.compile()
    return nc


def _get_nc():
    if "nc" not in _CACHE:
        _CACHE["nc"] = _build_nc()
    return _CACHE["nc"]


def make_in_maps(inputs, A, B, C, h0):
    u = np.ascontiguousarray(np.asarray(inputs, dtype=np.float32))
    A2 = np.ascontiguousarray(np.asarray(A, np.float32).reshape(2, 128).T)
    h02 = np.ascontiguousarray(np.asarray(h0, np.float32).reshape(2, 128).T)
    Br = np.ascontiguousarray(np.asarray(B, np.float32).reshape(2, 128, D))
    Cr = np.ascontiguousarray(np.asarray(C, np.float32).reshape(2, 128, D))
    return [
        {
            "u": np.ascontiguousarray(u[c * BLOC : (c + 1) * BLOC]),
            "A": A2,
            "B": Br,
            "C": Cr,
            "h0": h02,
        }
        for c in range(NCORES)
    ]


def kernel(inputs, A, B, C, h0, _trace=False):
    nc = _get_nc()
    in_maps = make_in_maps(inputs, A, B, C, h0)
    res = bass_utils.run_bass_kernel_spmd(
        nc, in_maps, core_ids=list(range(NCORES)), trace=_trace
    )
    out = np.concatenate([r["y"] for r in res.results], axis=0)
    if _trace:
        _CACHE["last_result"] = res
    return out
